# revision 21
# baseline (speedup 1.0000x reference)
"""Trainium2 Bass kernel for the pairwise-cosine masked ratio loss.

reference semantics:
    g  = min-max-normalized grad rows          [B, D]
    cos_g, cos_x = pairwise cosine Gram matrices
    loss = sum over same-class pairs i<j of (1-cos_g)/(1-cos_x) / B

Key facts used:
  * cosine is invariant to positive per-row affine scale, so min-max
    normalization reduces to u = (g - rowmin(g)) * S / ||g - rowmin(g)||
    with S = 32 (so dg = <u_i,u_j> = S^2 cos_g).
  * u is quantized to fp8e4 for the Gram matmuls.  Direct RNE
    quantization of the all-positive u rows has a per-coordinate mean
    bias that, over D=1024 dims, inflates every cosine by ~2e-3 (a
    +1.8% systematic error on the loss).  Fix: quantize the CENTERED
    rows w = u - mean(u) (symmetric density -> negligible bias, and
    sum(w)=0 kills the common-mode coupling), and restore the mean as
    an exact f32 rank-1 term: dg = <w_i,w_j> + M_i*M_j, M = sum(u)/32.
    Measured end-to-end error vs f64 reference: ~3e-4.
  * x needs no min-max shift, so its normalization is fully deferred:
    x̂ = fp8(x) is shipped host-pre-transposed; the device computes
    Gram-diagonal blocks, extracts ||x̂_j||^2, and scales
    cos_x = dx * b_i * b_j with b = 1/||x̂|| on the fly.
  * rows are sorted by class, class order chosen to minimize the max
    per-core RIGHT span; core k owns sorted rows [512k, 512k+512) and
    computes all same-class pairs (i, j) with i owned and j > i — each
    unordered pair is produced by exactly one core (host divides by B).

Device program (SPMD, identical program for all cores; data differs):
  x path:   DMA x̂^T tiles; per 128-col block: fp8 Gram diag block ->
            extract diagonal (stt vs identity, accum); sqrt+recip;
            tiny PE transposes + K=1 ones-matmul replicate into
            bxrow [128, ncol].
  g path:   per 128-row tile: row min (DVE, fused negate); Square
            activation bias=-min + accum -> ssq (Scalar); u_bf16 =
            (raw - min) * (S/||.||) with accum -> usum (DVE);
            w_fp8 = u - usum/D (Scalar, Identity w/ AP bias);
            PE-transpose into W^T [128, KT, ncol]; PSUM->SBUF copies
            on GpSimd.  M-row replicated like bxrow.
  phase 2:  per 128-row m-tile, column segs >= own offset (triangle):
            dgw  = fp8 DoubleRow matmuls (4 k-pairs, f32 PSUM)
            dx   = same on x̂
            t    = Mrow * M_i + dgw              (GpSimd stt)
            cx   = (dx * b_i) * bxrow            (GpSimd stt)
            sx   = min(cx - 1, -1e-30)           (GpSimd)
            num  = (t - S^2) * mask              (DVE stt)
            part += sum(num * recip_fast(sx))    (DVE)
  finale:   partition-reduce partials via matmul with ones -> [1,1].
Host sums the 8 partial scalars; loss = total / S^2 / B.
"""

import numpy as np

import concourse.bass as bass
import concourse.bacc as bacc
import concourse.mybir as mybir
import concourse.tile as tile
from concourse import bass_utils

B = 4096
D = 1024
NCORES = 8
NR = B // NCORES          # 512 own rows per core
KT = D // 128             # k-tiles
MT = NR // 128            # m-tiles per core
F32 = mybir.dt.float32
BF16 = mybir.dt.bfloat16
FP8 = mybir.dt.float8e4
S = 32.0                  # fp8 row scale for g; dg = S^2 * cos_g
S2 = float(S * S)
RD = float(np.sqrt(D))    # sqrt(1024) = 32
# device partials are S^2 * loss-sum; test.py computes
# total * OUT_SCALE / 2 / B, so fold 1/S^2 here.
OUT_SCALE = 2.0 / S2
AF = mybir.ActivationFunctionType
ALU = mybir.AluOpType
AX = mybir.AxisListType
DR = mybir.MatmulPerfMode.DoubleRow


def _segs(ncol: int, mi: int):
    """Column segments for m-tile mi: [128*mi, ncol) split into <=512
    chunks (balanced, 128-aligned)."""
    cs = mi * 128
    rem = ncol - cs
    segs = []
    while rem > 512:
        w = ((rem + 1) // 2 + 127) // 128 * 128
        w = min(w, 512)
        segs.append((cs, w))
        cs += w
        rem -= w
    segs.append((cs, rem))
    return segs


def _build_program(ncol: int, dbg: bool = False) -> bacc.Bacc:
    nc = bacc.Bacc("TRN2", target_bir_lowering=False, debug=False,
                   num_devices=NCORES)
    graw_d = nc.dram_tensor("graw", [ncol, D], BF16, kind="ExternalInput")
    xt_d = nc.dram_tensor("xt", [D, ncol], FP8, kind="ExternalInput")
    maskd = nc.dram_tensor("mask", [NR, ncol], FP8, kind="ExternalInput")
    ident_d = nc.dram_tensor("ident", [128, 128], FP8, kind="ExternalInput")
    identb_d = nc.dram_tensor("identb", [128, 128], BF16,
                              kind="ExternalInput")
    outd = nc.dram_tensor("out", [1, 1], F32, kind="ExternalOutput")

    NB = ncol // 128
    mi_segs = [_segs(ncol, mi) for mi in range(MT)]
    npart = sum(len(s) for s in mi_segs)
    if dbg:
        dbg_d = {
            "d_dsq": nc.dram_tensor("d_dsq", [128, NB], F32,
                                    kind="ExternalOutput"),
            "d_invq": nc.dram_tensor("d_invq", [128, NB], F32,
                                     kind="ExternalOutput"),
            "d_usum": nc.dram_tensor("d_usum", [128, NB], F32,
                                     kind="ExternalOutput"),
            "d_msc": nc.dram_tensor("d_msc", [128, NB], F32,
                                    kind="ExternalOutput"),
            "d_bxrow": nc.dram_tensor("d_bxrow", [128, ncol], BF16,
                                      kind="ExternalOutput"),
            "d_mrow": nc.dram_tensor("d_mrow", [128, ncol], BF16,
                                     kind="ExternalOutput"),
            "d_parts": nc.dram_tensor("d_parts", [128, npart], F32,
                                      kind="ExternalOutput"),
            "d_utg": nc.dram_tensor("d_utg", [128, KT, 128], FP8,
                                    kind="ExternalOutput"),
            "d_utx": nc.dram_tensor("d_utx", [128, KT, 128], FP8,
                                    kind="ExternalOutput"),
        }
    with tile.TileContext(nc) as tc:
        with (
            tc.tile_pool(name="cst", bufs=1) as cst,
            tc.tile_pool(name="io", bufs=4) as io,
            tc.tile_pool(name="ut", bufs=1) as utp,
            tc.tile_pool(name="sm", bufs=2) as smp,
            tc.tile_pool(name="wk", bufs=3) as wk,
            tc.tile_pool(name="tp", bufs=2, space="PSUM") as psp,
            tc.tile_pool(name="gr", bufs=2, space="PSUM") as psg,
            tc.tile_pool(name="fi", bufs=1, space="PSUM") as psf,
        ):
            identt = cst.tile([128, 128], FP8, name="identt")
            nc.sync.dma_start(identt[:], ident_d[:])
            identb = cst.tile([128, 128], BF16, name="identb")
            nc.sync.dma_start(identb[:], identb_d[:])
            onesk = cst.tile([1, 128], BF16, name="onesk")
            nc.vector.memset(onesk[:], 1.0)
            parts = cst.tile([128, npart], F32, name="parts")
            utg = utp.tile([128, KT, ncol], FP8, name="utg")
            utx = utp.tile([128, KT, ncol], FP8, name="utx")

            def replicate_row(src_col, dst_row, tag):
                """src_col [128, NB] bf16 -> dst_row [128, ncol] bf16
                (dst[p, b*128+q] = src[q, b] for every p)."""
                rowp = psp.tile([1, ncol], BF16, tag="bxr", name="rowp",
                                bufs=1)
                for b in range(NB):
                    nc.tensor.transpose(rowp[0:1, b * 128:(b + 1) * 128],
                                        src_col[:, b:b + 1], identb[:])
                srow = smp.tile([1, ncol], BF16, tag=f"s{tag}", name="srow")
                nc.scalar.copy(srow[:], rowp[0:1, :])
                cs = 0
                while cs < ncol:
                    cw = min(512, ncol - cs)
                    rep = psg.tile([128, 512], F32, tag="pg", name="rep")
                    nc.tensor.matmul(rep[:, :cw], onesk[:],
                                     srow[0:1, cs:cs + cw])
                    nc.vector.tensor_copy(dst_row[:, cs:cs + cw],
                                          rep[:, :cw])
                    cs += cw

            # ---- x path: load pre-transposed x̂, norms via Gram diag ----
            for k in range(KT):
                nc.sync.dma_start(utx[:, k, :], xt_d[k * 128:(k + 1) * 128, :])
            dsq = smp.tile([128, NB], F32, tag="dsq", name="dsq")
            for b in range(NB):
                pd = psg.tile([128, 512], F32, tag="px", name="pd")
                for k in range(KT):
                    nc.tensor.matmul(
                        pd[:, :128],
                        utx[:, k, b * 128:(b + 1) * 128],
                        utx[:, k, b * 128:(b + 1) * 128],
                        start=(k == 0), stop=(k == KT - 1))
                junkd = wk.tile([128, 128], F32, tag="junkd", name="junkd")
                nc.vector.scalar_tensor_tensor(
                    junkd[:], pd[:, :128], 1.0, identt[:],
                    op0=ALU.mult, op1=ALU.mult,
                    accum_out=dsq[:, b:b + 1])
            nrmq = smp.tile([128, NB], F32, tag="nrmq", name="nrmq")
            nc.scalar.sqrt(nrmq[:], dsq[:])
            invq = smp.tile([128, NB], F32, tag="invq", name="invq")
            nc.vector.reciprocal(invq[:], nrmq[:])
            invqb = smp.tile([128, NB], BF16, tag="invqb", name="invqb")
            nc.gpsimd.tensor_copy(invqb[:], invq[:])
            bxrow = cst.tile([128, ncol], BF16, name="bxrow")
            replicate_row(invqb, bxrow, "bx")

            # ---- g path: normalize, center, fp8 + transpose ----
            nm = smp.tile([128, NB], F32, tag="nm", name="nm")
            ssq = smp.tile([128, NB], F32, tag="ssq", name="ssq")
            invS = smp.tile([128, NB], F32, tag="invS", name="invS")
            vsum = smp.tile([128, NB], F32, tag="vsum", name="vsum")
            usum = smp.tile([128, NB], F32, tag="usum", name="usum")
            mneg = smp.tile([128, NB], F32, tag="mneg", name="mneg")
            msc = smp.tile([128, NB], F32, tag="msc", name="msc")
            groups = [list(range(0, (NB + 1) // 2)),
                      list(range((NB + 1) // 2, NB))]
            for grp in groups:
                raws = {}
                for t in grp:
                    raw = io.tile([128, D], BF16, tag="raw", name="raw")
                    nc.sync.dma_start(raw[:],
                                      graw_d[t * 128:(t + 1) * 128, :])
                    raws[t] = raw
                    nc.vector.tensor_reduce(nm[:, t:t + 1], raw[:],
                                            axis=AX.X, op=ALU.min,
                                            negate=True)
                    sq = wk.tile([128, D], BF16, tag="sq", name="sq")
                    nc.scalar.activation(sq[:], raw[:], AF.Square,
                                         bias=nm[:, t:t + 1], scale=1.0,
                                         accum_out=ssq[:, t:t + 1])
                g0, gn = grp[0], len(grp)
                gsl = slice(g0, g0 + gn)
                nrm = smp.tile([128, NB], F32, tag="nrm", name="nrm")
                nc.scalar.sqrt(nrm[:, gsl], ssq[:, gsl])
                inv = smp.tile([128, NB], F32, tag="inv", name="inv")
                nc.vector.reciprocal(inv[:, gsl], nrm[:, gsl])
                nc.gpsimd.tensor_scalar_mul(invS[:, gsl], inv[:, gsl], S)
                for t in grp:
                    # v = raw - min (accum: vsum); scale applied in w-op
                    v = wk.tile([128, D], BF16, tag="u", name="v")
                    nc.vector.tensor_scalar(v[:], raws[t][:],
                                            nm[:, t:t + 1], None,
                                            op0=ALU.add, op1=ALU.add,
                                            accum_out=vsum[:, t:t + 1])
                    # usum = vsum*invS; mneg = -usum/D ; msc = usum/sqrt(D)
                    nc.gpsimd.tensor_tensor(usum[:, t:t + 1],
                                            vsum[:, t:t + 1],
                                            invS[:, t:t + 1], op=ALU.mult)
                    nc.gpsimd.tensor_scalar_mul(mneg[:, t:t + 1],
                                                usum[:, t:t + 1], -1.0 / D)
                    nc.gpsimd.tensor_scalar_mul(msc[:, t:t + 1],
                                                usum[:, t:t + 1], 1.0 / RD)
                    w = wk.tile([128, D], BF16, tag="w", name="w")
                    nc.scalar.activation(w[:], v[:], AF.Identity,
                                         bias=mneg[:, t:t + 1],
                                         scale=invS[:, t:t + 1])
                    ps = psp.tile([128, D], BF16, tag="tp", name="ps")
                    for kk in range(KT):
                        nc.tensor.transpose(
                            ps[:, kk * 128:(kk + 1) * 128],
                            w[:, kk * 128:(kk + 1) * 128],
                            identb[:])
                    cp = nc.scalar.copy if t % 2 == 0 else \
                        nc.vector.tensor_copy
                    cp(
                        utg[:, :, t * 128:(t + 1) * 128],
                        ps[:].rearrange("p (k c) -> p k c", k=KT),
                    )
            # M row replicated
            mscb = smp.tile([128, NB], BF16, tag="mscb", name="mscb")
            nc.gpsimd.tensor_copy(mscb[:], msc[:])
            mrow = cst.tile([128, ncol], BF16, name="mrow")
            replicate_row(mscb, mrow, "m")

            # ---- phase 2: Gram blocks + masked ratio ----
            pidx = 0
            for mi in range(MT):
                maskt = wk.tile([128, ncol], FP8, tag="maskt", name="maskt",
                                bufs=2)
                nc.sync.dma_start(maskt[:],
                                  maskd[mi * 128:(mi + 1) * 128, :])
                for cs, cw in mi_segs[mi]:
                    pg = psg.tile([128, 512], F32, tag="pg", name="pg")
                    px = psg.tile([128, 512], F32, tag="px", name="px")
                    for k2 in range(KT // 2):
                        nc.tensor.matmul(
                            pg[:, :cw],
                            utg[:, 2 * k2:2 * k2 + 2,
                                mi * 128:(mi + 1) * 128],
                            utg[:, 2 * k2:2 * k2 + 2, cs:cs + cw],
                            start=(k2 == 0), stop=(k2 == KT // 2 - 1),
                            perf_mode=DR)
                    for k2 in range(KT // 2):
                        nc.tensor.matmul(
                            px[:, :cw],
                            utx[:, 2 * k2:2 * k2 + 2,
                                mi * 128:(mi + 1) * 128],
                            utx[:, 2 * k2:2 * k2 + 2, cs:cs + cw],
                            start=(k2 == 0), stop=(k2 == KT // 2 - 1),
                            perf_mode=DR)
                    tg = wk.tile([128, 512], F32, tag="tg", name="tg")
                    nc.vector.scalar_tensor_tensor(
                        tg[:, :cw], mrow[:, cs:cs + cw],
                        msc[:, mi:mi + 1], pg[:, :cw],
                        op0=ALU.mult, op1=ALU.add)
                    cx = wk.tile([128, 512], F32, tag="cx", name="cx")
                    nc.vector.scalar_tensor_tensor(
                        cx[:, :cw], px[:, :cw], invq[:, mi:mi + 1],
                        bxrow[:, cs:cs + cw], op0=ALU.mult, op1=ALU.mult)
                    sx = wk.tile([128, 512], F32, tag="sx", name="sx")
                    nc.gpsimd.tensor_scalar(sx[:, :cw], cx[:, :cw], 1.0,
                                            -1e-30, op0=ALU.subtract,
                                            op1=ALU.min)
                    rx = wk.tile([128, 512], F32, tag="rx", name="rx")
                    nc.vector.reciprocal_approx_fast(rx[:, :cw], sx[:, :cw])
                    t2 = wk.tile([128, 512], F32, tag="t2", name="t2")
                    nc.gpsimd.tensor_scalar(t2[:, :cw], tg[:, :cw], S2,
                                            None, op0=ALU.subtract)
                    num = wk.tile([128, 512], F32, tag="num", name="num")
                    nc.gpsimd.tensor_tensor(num[:, :cw], t2[:, :cw],
                                            maskt[:, cs:cs + cw],
                                            op=ALU.mult)
                    junk = wk.tile([128, 512], F32, tag="junk", name="junk")
                    nc.vector.scalar_tensor_tensor(
                        junk[:, :cw], num[:, :cw], 1.0, rx[:, :cw],
                        op0=ALU.mult, op1=ALU.mult,
                        accum_out=parts[:, pidx:pidx + 1])
                    pidx += 1

            if dbg:
                nc.sync.dma_start(dbg_d["d_dsq"][:], dsq[:])
                nc.sync.dma_start(dbg_d["d_invq"][:], invq[:])
                nc.sync.dma_start(dbg_d["d_usum"][:], usum[:])
                nc.sync.dma_start(dbg_d["d_msc"][:], msc[:])
                nc.sync.dma_start(dbg_d["d_bxrow"][:], bxrow[:])
                nc.sync.dma_start(dbg_d["d_mrow"][:], mrow[:])
                nc.sync.dma_start(dbg_d["d_parts"][:], parts[:, :npart])
                nc.sync.dma_start(dbg_d["d_utg"][:], utg[:, :, 0:128])
                nc.sync.dma_start(dbg_d["d_utx"][:], utx[:, :, 0:128])

            # ---- finale: reduce partials to one scalar ----
            total = smp.tile([128, 1], F32, tag="total", name="total")
            nc.vector.reduce_sum(total[:], parts[:], axis=AX.X)
            ones = cst.tile([128, 1], F32, name="ones")
            nc.vector.memset(ones[:], 1.0)
            fin = psf.tile([1, 1], F32, name="fin")
            nc.tensor.matmul(fin[:], total[:], ones[:])
            outs = smp.tile([1, 1], F32, tag="outs", name="outs")
            nc.scalar.copy(outs[:], fin[:])
            nc.sync.dma_start(outd[:], outs[:])

    nc.compile()
    return nc


_PROGRAM_CACHE: dict = {}


def _get_program(ncol: int) -> bacc.Bacc:
    if ncol not in _PROGRAM_CACHE:
        _PROGRAM_CACHE[ncol] = _build_program(ncol)
    return _PROGRAM_CACHE[ncol]


def _choose_order(sizes: np.ndarray, nsamples: int = 40000) -> np.ndarray:
    """Pick a class ordering minimizing the max per-core RIGHT span
    (own 512 rows + overhang of the class crossing the core's end)."""
    ncls = len(sizes)
    rng = np.random.default_rng(0)
    perms = np.empty((nsamples + 2, ncls), dtype=np.int64)
    perms[0] = np.arange(ncls)
    perms[1] = np.argsort(sizes)[::-1]
    for i in range(nsamples):
        perms[i + 2] = rng.permutation(ncls)
    s = sizes[perms]                                   # [N, ncls]
    pref = np.concatenate(
        [np.zeros((len(perms), 1), np.int64), np.cumsum(s, axis=1)], axis=1)
    worst = np.zeros(len(perms), dtype=np.int64)
    for k in range(NCORES):
        r0, r1 = k * NR, (k + 1) * NR
        ci = (pref[:, 1:-1] <= r1 - 1).sum(axis=1)     # class holding r1-1
        hi = np.take_along_axis(pref, (ci + 1)[:, None], axis=1)[:, 0]
        span = np.maximum(hi, r1) - r0
        worst = np.maximum(worst, span)
    return perms[int(np.argmin(worst))]


def _prep_host(outputs: np.ndarray, grad: np.ndarray, x: np.ndarray):
    """Class sort, per-core right-span column blocks, masks, fp8 x̂^T."""
    bf = mybir.dt.np(BF16)
    f8 = mybir.dt.np(FP8)
    g = grad.reshape(B, -1).astype(bf)
    xq = x.reshape(B, -1).astype(f8)
    cls = np.argmax(outputs, axis=1)
    ncls = outputs.shape[1]
    sizes = np.bincount(cls, minlength=ncls)
    order = _choose_order(sizes)

    perm = np.concatenate([np.nonzero(cls == c)[0] for c in order])
    pcls = cls[perm]
    pref = np.concatenate([[0], np.cumsum(sizes[order])])

    spans = []
    for k in range(NCORES):
        r0, r1 = k * NR, (k + 1) * NR
        ci = int(np.searchsorted(pref, r1 - 1, side="right")) - 1
        hi = max(int(pref[ci + 1]), r1)
        spans.append((r0, hi))
    ncol = ((max(hi - r0 for r0, hi in spans) + 127) // 128) * 128

    ident = np.eye(128, dtype=f8)
    identb = np.eye(128, dtype=bf)
    in_maps = []
    for k in range(NCORES):
        r0, hi = spans[k]
        nreal = hi - r0
        cols = np.concatenate(
            [np.arange(r0, hi),
             np.repeat([hi - 1], ncol - nreal)])
        rows_global = perm[cols]                       # original row ids
        graw = g[rows_global]                          # [ncol, D] bf16
        xt = np.ascontiguousarray(xq[rows_global].T)   # [D, ncol] fp8
        rowcls = pcls[r0:r0 + NR]
        colcls = np.full(ncol, -1, dtype=np.int64)
        colcls[:nreal] = pcls[cols[:nreal]]
        gi = np.arange(r0, r0 + NR)[:, None]
        gj = np.full(ncol, -2, dtype=np.int64)
        gj[:nreal] = cols[:nreal]
        mask = ((rowcls[:, None] == colcls[None, :])
                & (gi < gj[None, :])).astype(np.float32)
        in_maps.append({
            "graw": np.ascontiguousarray(graw),
            "xt": xt,
            "mask": np.ascontiguousarray(mask.astype(f8)),
            "ident": ident,
            "identb": identb,
        })
    return ncol, in_maps


def kernel(outputs, grad, x, y):
    outputs = np.asarray(outputs)
    grad = np.asarray(grad)
    x = np.asarray(x)
    ncol, in_maps = _prep_host(outputs, grad, x)
    nc = _get_program(ncol)
    res = bass_utils.run_bass_kernel_spmd(
        nc, in_maps, core_ids=list(range(NCORES)))
    total = float(sum(r["out"][0, 0].astype(np.float64)
                      for r in res.results))
    loss = total * OUT_SCALE / 2.0 / float(B)
    return np.float32(loss)
